# revision 19
# baseline (speedup 1.0000x reference)
"""Trainium2 Bass kernel for nn_AttentionModel (B=4, S=2048, H=16, D=128).

Multi-head attention: QKV linear projections -> scaled dot-product
softmax -> deterministic dropout (threefry, key 42) -> PV matmul.

Sharding: the 64 (b, h) attention slices are embarrassingly parallel;
core c computes slices g in [8c, 8c+8).  Projection weights replicated.

Device-side layout is "T layout" (head-dim on partitions, sequence on
free) so no on-chip transposes are needed anywhere:
  - host ships x^T per slice; qT/kT computed as [d', s]
  - scoresT[j, i] = sum_d kT[d,j] qT[d,i]  (f32r matmuls, 1 cyc/row)
  - exp on ACT (PSUM->SBUF, bf16 out), denominator = column sums of
    exp via DVE pair-adds + one PE ones-matmul (the "ones" hold 0.9,
    folding the dropout 1/(1-p) factor into the reciprocal)
  - dropout as one big DVE bf16 multiply with the {0,1} mask
  - out^T[d, i] = sum_j v[j,d] tmask[j,i] (bf16 matmuls), scaled by
    the broadcast reciprocal on DVE; host un-transposes.
"""

import sys

sys.path.insert(0, "/opt/trn_rl_repo")

import numpy as np

B, S, H, D = 4, 2048, 16, 128
N_CORES = 8
SLICES_PER_CORE = (B * H) // N_CORES  # 8
DROPOUT_P = 0.1
SCALE = 1.0 / np.sqrt(np.float32(D))

IB = 512              # i-block width
N_IB = S // IB        # 4 i-blocks per slice
NJ = S // 128         # 16 j-tiles
EXP_GROUP = 2         # j-tiles per exp op (PSUM banks per scores tile)

_PROGRAM = None


def _build_program():
    import concourse.bacc as bacc
    import concourse.tile as tile
    from concourse import mybir

    f32 = mybir.dt.float32
    f16 = mybir.dt.float16
    bf16 = mybir.dt.bfloat16
    Exp = mybir.ActivationFunctionType.Exp
    Identity = mybir.ActivationFunctionType.Identity
    mult = mybir.AluOpType.mult
    add = mybir.AluOpType.add

    nc = bacc.Bacc("TRN2", target_bir_lowering=False, debug=False,
                   num_devices=N_CORES)

    P = SLICES_PER_CORE
    xq_d = nc.dram_tensor("xqT", [P, D, S], f16, kind="ExternalInput")
    xk_d = nc.dram_tensor("xkT", [P, D, S], f16, kind="ExternalInput")
    xv_d = nc.dram_tensor("xvT", [P, D, S], f16, kind="ExternalInput")
    wq_d = nc.dram_tensor("wqT", [D, D], f16, kind="ExternalInput")
    wk_d = nc.dram_tensor("wkT", [D, D], f16, kind="ExternalInput")
    wv_d = nc.dram_tensor("wvT", [D, D], f16, kind="ExternalInput")
    bq_d = nc.dram_tensor("bq", [D, 1], f16, kind="ExternalInput")
    bk_d = nc.dram_tensor("bk", [D, 1], f16, kind="ExternalInput")
    bv_d = nc.dram_tensor("bv", [1, D], f16, kind="ExternalInput")
    ones_d = nc.dram_tensor("ones_row", [1, IB], f16, kind="ExternalInput")
    mask_d = nc.dram_tensor("maskT", [P, S, S], f16, kind="ExternalInput")
    out_d = nc.dram_tensor("outT", [P, D, S], f32, kind="ExternalOutput")

    with tile.TileContext(nc) as tc:
        with (
            tc.tile_pool(name="consts", bufs=1) as consts,
            tc.tile_pool(name="xpool", bufs=3) as xpool,
            tc.tile_pool(name="qk", bufs=2) as qkpool,
            tc.tile_pool(name="vpool", bufs=2) as vpool,
            tc.tile_pool(name="tpool", bufs=3) as tpool,
            tc.tile_pool(name="mpool", bufs=3) as mpool,
            tc.tile_pool(name="tmask", bufs=2) as tmpool,
            tc.tile_pool(name="dtree", bufs=2) as dtree,
            tc.tile_pool(name="small", bufs=2) as small,
            tc.tile_pool(name="outp", bufs=2) as outp,
            tc.tile_pool(name="ps_scores", bufs=2, space="PSUM") as ps_scores,
            tc.tile_pool(name="ps_out", bufs=2, space="PSUM") as ps_out,
            tc.tile_pool(name="ps_aux", bufs=2, space="PSUM") as ps_aux,
        ):
            # --- constants ---
            wq_sb = consts.tile([D, D], f16, tag="wq")
            wk_sb = consts.tile([D, D], f16, tag="wk")
            wv_sb = consts.tile([D, D], f16, tag="wv")
            bq_sb = consts.tile([D, 1], f16, tag="bq")
            bk_sb = consts.tile([D, 1], f16, tag="bk")
            bv_sb = consts.tile([1, D], f16, tag="bv")
            ones_row = consts.tile([1, IB], f16, tag="ones_row")
            ones_bf = consts.tile([128, 128], f16, tag="ones_bf")
            nc.sync.dma_start(out=wq_sb[:], in_=wq_d[:])
            nc.sync.dma_start(out=wk_sb[:], in_=wk_d[:])
            nc.sync.dma_start(out=wv_sb[:], in_=wv_d[:])
            nc.sync.dma_start(out=bq_sb[:], in_=bq_d[:])
            nc.sync.dma_start(out=bk_sb[:], in_=bk_d[:])
            nc.sync.dma_start(out=bv_sb[:], in_=bv_d[:])
            nc.sync.dma_start(out=ones_row[:], in_=ones_d[:])
            nc.vector.memset(ones_bf[:], 1.0)

            for s in range(P):
                # ---- load inputs (T layout) ----
                xq_s = xpool.tile([D, S], f16, tag="x")
                xk_s = xpool.tile([D, S], f16, tag="x")
                xv_s = xpool.tile([D, S], f16, tag="x")
                nc.sync.dma_start(out=xq_s[:], in_=xq_d[s])
                nc.sync.dma_start(out=xk_s[:], in_=xk_d[s])
                nc.sync.dma_start(out=xv_s[:], in_=xv_d[s])

                # ---- projections: qT/kT = W @ xT + b (bias via K=1 matmul) ----
                qT = qkpool.tile([D, S], f16, tag="qT")
                kT = qkpool.tile([D, S], f16, tag="kT")
                for ib in range(N_IB):
                    sl = slice(ib * IB, (ib + 1) * IB)
                    pq = ps_aux.tile([128, IB], f32, tag="aux")
                    nc.tensor.matmul(pq[:], lhsT=wq_sb[:], rhs=xq_s[:, sl],
                                     start=True, stop=True)
                    nc.scalar.activation(qT[:, sl], pq[:], Identity,
                                         bias=bq_sb[:])
                    pk = ps_aux.tile([128, IB], f32, tag="aux")
                    nc.tensor.matmul(pk[:], lhsT=wk_sb[:], rhs=xk_s[:, sl],
                                     start=True, stop=True)
                    nc.scalar.activation(kT[:, sl], pk[:], Identity,
                                         bias=bk_sb[:])

                # ---- v natural [j, d'] in bf16 ----
                v_sb = vpool.tile([128, NJ, D], f16, tag="v")
                for jn in range(NJ):
                    jsl = slice(jn * 128, (jn + 1) * 128)
                    pv = ps_aux.tile([128, IB], f32, tag="aux")
                    nc.tensor.matmul(pv[:, 0:D], lhsT=xv_s[:, jsl],
                                     rhs=wv_sb[:], start=True, stop=False)
                    nc.tensor.matmul(pv[:, 0:D], lhsT=ones_row[:, 0:128],
                                     rhs=bv_sb[:], start=False, stop=True)
                    nc.vector.tensor_copy(v_sb[:, jn, :], pv[:, 0:D])

                outT = outp.tile([D, S], f32, tag="outT")

                for ib in range(N_IB):
                    isl = slice(ib * IB, (ib + 1) * IB)
                    # ---- scoresT + exp ----
                    t_buf = tpool.tile([128, NJ, IB], f16, tag="t")
                    for g in range(NJ // EXP_GROUP):
                        ps = ps_scores.tile([128, EXP_GROUP, IB], f32, tag="sc")
                        for u in range(EXP_GROUP):
                            jn = g * EXP_GROUP + u
                            jsl = slice(jn * 128, (jn + 1) * 128)
                            nc.tensor.matmul(ps[:, u, :], lhsT=kT[:, jsl],
                                             rhs=qT[:, isl],
                                             start=True, stop=True)
                        nc.scalar.activation(
                            t_buf[:, g * EXP_GROUP:(g + 1) * EXP_GROUP, :],
                            ps[:], Exp, scale=float(SCALE))

                    # ---- denominator: sum_j t (DVE pair-add + PE ones-mm) ----
                    dt1 = dtree.tile([128, NJ // 2, IB], f16, tag="dt")
                    nc.vector.tensor_tensor(out=dt1[:, 0:4, :],
                                            in0=t_buf[:, 0:4, :],
                                            in1=t_buf[:, 8:12, :], op=add)
                    nc.vector.tensor_tensor(out=dt1[:, 4:8, :],
                                            in0=t_buf[:, 4:8, :],
                                            in1=t_buf[:, 12:16, :], op=add)
                    nc.vector.tensor_tensor(out=dt1[:, 0:4, :],
                                            in0=dt1[:, 0:4, :],
                                            in1=dt1[:, 4:8, :], op=add)
                    pd = ps_aux.tile([128, IB], f32, tag="aux")
                    for m in range(NJ // 4):
                        nc.tensor.matmul(pd[:], lhsT=ones_bf[:],
                                         rhs=dt1[:, m, :],
                                         start=(m == 0), stop=(m == NJ // 4 - 1))
                    recip = small.tile([128, IB], f32, tag="recip")
                    nc.vector.reciprocal_approx_fast(out=recip[:], in_=pd[:])

                    # ---- dropout: t *= mask ----
                    m_buf = mpool.tile([128, NJ, IB], f16, tag="m")
                    nc.sync.dma_start(
                        out=m_buf[:],
                        in_=mask_d[s, :, isl].rearrange(
                            "(jn jp) i -> jp jn i", jp=128))
                    tm_buf = tmpool.tile([128, NJ, IB], f16, tag="tm")
                    nc.vector.tensor_tensor(out=tm_buf[:, 0:8, :],
                                            in0=t_buf[:, 0:8, :],
                                            in1=m_buf[:, 0:8, :], op=mult)
                    nc.vector.tensor_tensor(out=tm_buf[:, 8:16, :],
                                            in0=t_buf[:, 8:16, :],
                                            in1=m_buf[:, 8:16, :], op=mult)

                    # ---- PV: outT[d, i] = sum_j v[j,d] tmask[j,i] ----
                    po = ps_out.tile([128, IB], f32, tag="po")
                    for jn in range(NJ):
                        nc.tensor.matmul(po[:], lhsT=v_sb[:, jn, :],
                                         rhs=tm_buf[:, jn, :],
                                         start=(jn == 0), stop=(jn == NJ - 1))
                    nc.vector.tensor_tensor(out=outT[:, isl], in0=po[:],
                                            in1=recip[:], op=mult)

                nc.sync.dma_start(out=out_d[s], in_=outT[:])

    nc.finalize()
    return nc


def _get_program():
    global _PROGRAM
    if _PROGRAM is None:
        _PROGRAM = _build_program()
    return _PROGRAM


def _dropout_mask():
    """Exact replica of the reference's threefry dropout keep-mask."""
    import jax

    with jax.default_device(jax.devices("cpu")[0]):
        keep = jax.random.bernoulli(jax.random.key(42), 1.0 - DROPOUT_P,
                                    (B, H, S, S))
        return np.asarray(keep)


def _prepare_in_maps(query, key, value, Wq, bq, Wk, bk, Wv, bv):
    import ml_dtypes

    query = np.asarray(query, dtype=np.float32)
    key = np.asarray(key, dtype=np.float32)
    value = np.asarray(value, dtype=np.float32)
    Wq = np.asarray(Wq, dtype=np.float32)
    Wk = np.asarray(Wk, dtype=np.float32)
    Wv = np.asarray(Wv, dtype=np.float32)
    bq = np.asarray(bq, dtype=np.float32).reshape(D, 1)
    bk = np.asarray(bk, dtype=np.float32).reshape(D, 1)
    bv = np.asarray(bv, dtype=np.float32).reshape(1, D)

    keep = _dropout_mask()  # [B, H, S(i), S(j)] bool

    # per-slice transposed views: x[g] = input[b, :, h, :].T  -> [D, S]
    # g = b*H + h ; core c gets g in [8c, 8c+8)
    def xt(x):  # [B, S, H, D] -> [B*H, D, S] in fp16
        return np.ascontiguousarray(
            x.transpose(0, 2, 3, 1).reshape(B * H, D, S)).astype(np.float16)

    xqT, xkT, xvT = xt(query), xt(key), xt(value)
    maskT = np.ascontiguousarray(
        keep.transpose(0, 1, 3, 2).reshape(B * H, S, S)).astype(np.float16)

    ones_row_h = np.ones((1, IB), dtype=np.float16)
    wqT = np.ascontiguousarray(Wq.T).astype(np.float16)
    wkT = np.ascontiguousarray(Wk.T).astype(np.float16)
    # dropout 1/(1-p) folded into the V projection
    wvT = (np.ascontiguousarray(Wv.T) / (1.0 - DROPOUT_P)).astype(np.float16)
    bv = (bv / (1.0 - DROPOUT_P)).astype(np.float16)
    bq = bq.astype(np.float16)
    bk = bk.astype(np.float16)

    in_maps = []
    for c in range(N_CORES):
        g = slice(c * SLICES_PER_CORE, (c + 1) * SLICES_PER_CORE)
        in_maps.append({
            "xqT": xqT[g], "xkT": xkT[g], "xvT": xvT[g],
            "wqT": wqT, "wkT": wkT, "wvT": wvT,
            "bq": bq, "bk": bk, "bv": bv,
            "maskT": maskT[g],
            "ones_row": ones_row_h,
        })

    return in_maps


def _gather(results):
    out = np.empty((B, H, S, D), dtype=np.float32)
    for c in range(N_CORES):
        outT = results[c]["outT"]  # [8, D, S]
        for s in range(SLICES_PER_CORE):
            g = c * SLICES_PER_CORE + s
            out[g // H, g % H] = outT[s].T
    return out


def kernel(query, key, value, Wq, bq, Wk, bk, Wv, bv):
    from concourse.bass_utils import run_bass_kernel_spmd

    in_maps = _prepare_in_maps(query, key, value, Wq, bq, Wk, bk, Wv, bv)
    nc = _get_program()
    res = run_bass_kernel_spmd(nc, in_maps, list(range(N_CORES)))
    return _gather(res.results)


# revision 21
# speedup vs baseline: 1.1957x; 1.1957x over previous
"""Trainium2 Bass kernel for nn_AttentionModel (B=4, S=2048, H=16, D=128).

Multi-head attention: QKV linear projections -> scaled dot-product
softmax -> deterministic dropout (threefry, key 42) -> PV matmul.

Sharding: the 64 (b, h) attention slices are embarrassingly parallel;
core c computes slices g in [8c, 8c+8).  Projection weights replicated.

Device-side layout is "T layout" (head-dim on partitions, sequence on
free) so no on-chip transposes are needed anywhere:
  - host ships x^T per slice; qT/kT computed as [d', s]
  - scoresT[j, i] = sum_d kT[d,j] qT[d,i]  (f32r matmuls, 1 cyc/row)
  - exp on ACT (PSUM->SBUF, bf16 out), denominator = column sums of
    exp via DVE pair-adds + one PE ones-matmul (the "ones" hold 0.9,
    folding the dropout 1/(1-p) factor into the reciprocal)
  - dropout as one big DVE bf16 multiply with the {0,1} mask
  - out^T[d, i] = sum_j v[j,d] tmask[j,i] (bf16 matmuls), scaled by
    the broadcast reciprocal on DVE; host un-transposes.
"""

import sys

sys.path.insert(0, "/opt/trn_rl_repo")

import numpy as np

B, S, H, D = 4, 2048, 16, 128
N_CORES = 8
SLICES_PER_CORE = (B * H) // N_CORES  # 8
DROPOUT_P = 0.1
SCALE = 1.0 / np.sqrt(np.float32(D))

IB = 512              # i-block width
N_IB = S // IB        # 4 i-blocks per slice
NJ = S // 128         # 16 j-tiles
EXP_GROUP = 2         # j-tiles per exp op (PSUM banks per scores tile)

_PROGRAM = None


def _build_program():
    import concourse.bacc as bacc
    import concourse.tile as tile
    from concourse import mybir

    f32 = mybir.dt.float32
    f16 = mybir.dt.float16
    bf16 = mybir.dt.bfloat16
    Exp = mybir.ActivationFunctionType.Exp
    Identity = mybir.ActivationFunctionType.Identity
    mult = mybir.AluOpType.mult
    add = mybir.AluOpType.add

    nc = bacc.Bacc("TRN2", target_bir_lowering=False, debug=False,
                   num_devices=N_CORES)

    P = SLICES_PER_CORE
    xq_d = nc.dram_tensor("xqT", [P, D, S], f16, kind="ExternalInput")
    xk_d = nc.dram_tensor("xkT", [P, D, S], f16, kind="ExternalInput")
    xv_d = nc.dram_tensor("xvT", [P, D, S], f16, kind="ExternalInput")
    wq_d = nc.dram_tensor("wqT", [D, D], f16, kind="ExternalInput")
    wk_d = nc.dram_tensor("wkT", [D, D], f16, kind="ExternalInput")
    wv_d = nc.dram_tensor("wvT", [D, D], f16, kind="ExternalInput")
    bq_d = nc.dram_tensor("bq", [D, 1], f16, kind="ExternalInput")
    bk_d = nc.dram_tensor("bk", [D, 1], f16, kind="ExternalInput")
    bv_d = nc.dram_tensor("bv", [1, D], f16, kind="ExternalInput")
    ones_d = nc.dram_tensor("ones_row", [1, IB], f16, kind="ExternalInput")
    mask_d = nc.dram_tensor("maskT", [P, S, S], f16, kind="ExternalInput")
    out_d = nc.dram_tensor("outT", [P, D, S], f32, kind="ExternalOutput")

    with tile.TileContext(nc) as tc:
        with (
            tc.tile_pool(name="consts", bufs=1) as consts,
            tc.tile_pool(name="xpool", bufs=3) as xpool,
            tc.tile_pool(name="qk", bufs=2) as qkpool,
            tc.tile_pool(name="vpool", bufs=3) as vpool,
            tc.tile_pool(name="tpool", bufs=3) as tpool,
            tc.tile_pool(name="mpool", bufs=3) as mpool,
            tc.tile_pool(name="tmask", bufs=2) as tmpool,
            tc.tile_pool(name="dtree", bufs=2) as dtree,
            tc.tile_pool(name="small", bufs=2) as small,
            tc.tile_pool(name="outp", bufs=2) as outp,
            tc.tile_pool(name="ps_scores", bufs=2, space="PSUM") as ps_scores,
            tc.tile_pool(name="ps_denom", bufs=2, space="PSUM") as ps_denom,
            tc.tile_pool(name="ps_out", bufs=1, space="PSUM") as ps_out,
            tc.tile_pool(name="ps_proj", bufs=1, space="PSUM") as ps_proj,
        ):
            # --- constants ---
            wq_sb = consts.tile([D, D], f16, tag="wq")
            wk_sb = consts.tile([D, D], f16, tag="wk")
            wv_sb = consts.tile([D, D], f16, tag="wv")
            bq_sb = consts.tile([D, 1], f16, tag="bq")
            bk_sb = consts.tile([D, 1], f16, tag="bk")
            bv_sb = consts.tile([1, D], f16, tag="bv")
            ones_row = consts.tile([1, IB], f16, tag="ones_row")
            ones_bf = consts.tile([128, 128], f16, tag="ones_bf")
            nc.sync.dma_start(out=wq_sb[:], in_=wq_d[:])
            nc.sync.dma_start(out=wk_sb[:], in_=wk_d[:])
            nc.sync.dma_start(out=wv_sb[:], in_=wv_d[:])
            nc.sync.dma_start(out=bq_sb[:], in_=bq_d[:])
            nc.sync.dma_start(out=bk_sb[:], in_=bk_d[:])
            nc.sync.dma_start(out=bv_sb[:], in_=bv_d[:])
            nc.sync.dma_start(out=ones_row[:], in_=ones_d[:])
            nc.vector.memset(ones_bf[:], 1.0)

            for s in range(P):
                # ---- load inputs (T layout) ----
                xq_s = xpool.tile([D, S], f16, tag="x")
                xk_s = xpool.tile([D, S], f16, tag="x")
                xv_s = xpool.tile([D, S], f16, tag="x")
                nc.sync.dma_start(out=xq_s[:], in_=xq_d[s])
                nc.sync.dma_start(out=xk_s[:], in_=xk_d[s])
                nc.sync.dma_start(out=xv_s[:], in_=xv_d[s])

                # ---- projections: qT/kT = W @ xT + b (bias via K=1 matmul) ----
                qT = qkpool.tile([D, S], f16, tag="qT")
                kT = qkpool.tile([D, S], f16, tag="kT")
                for ib in range(N_IB):
                    sl = slice(ib * IB, (ib + 1) * IB)
                    pq = ps_proj.tile([128, IB], f32, tag="proj")
                    nc.tensor.matmul(pq[:], lhsT=wq_sb[:], rhs=xq_s[:, sl],
                                     start=True, stop=True)
                    nc.scalar.activation(qT[:, sl], pq[:], Identity,
                                         bias=bq_sb[:])
                    pk = ps_proj.tile([128, IB], f32, tag="proj")
                    nc.tensor.matmul(pk[:], lhsT=wk_sb[:], rhs=xk_s[:, sl],
                                     start=True, stop=True)
                    nc.scalar.activation(kT[:, sl], pk[:], Identity,
                                         bias=bk_sb[:])

                # ---- v natural [j, d'] in bf16 ----
                v_sb = vpool.tile([128, NJ, D], f16, tag="v")
                for jn in range(NJ):
                    jsl = slice(jn * 128, (jn + 1) * 128)
                    pv = ps_proj.tile([128, IB], f32, tag="proj")
                    nc.tensor.matmul(pv[:, 0:D], lhsT=xv_s[:, jsl],
                                     rhs=wv_sb[:], start=True, stop=False)
                    nc.tensor.matmul(pv[:, 0:D], lhsT=ones_row[:, 0:128],
                                     rhs=bv_sb[:], start=False, stop=True)
                    nc.vector.tensor_copy(v_sb[:, jn, :], pv[:, 0:D])

                outT = outp.tile([D, S], f32, tag="outT")

                for ib in range(N_IB):
                    isl = slice(ib * IB, (ib + 1) * IB)
                    # ---- scoresT + exp ----
                    t_buf = tpool.tile([128, NJ, IB], f16, tag="t")
                    for g in range(NJ // EXP_GROUP):
                        ps = ps_scores.tile([128, EXP_GROUP, IB], f32, tag="sc")
                        for u in range(EXP_GROUP):
                            jn = g * EXP_GROUP + u
                            jsl = slice(jn * 128, (jn + 1) * 128)
                            nc.tensor.matmul(ps[:, u, :], lhsT=kT[:, jsl],
                                             rhs=qT[:, isl],
                                             start=True, stop=True)
                        nc.scalar.activation(
                            t_buf[:, g * EXP_GROUP:(g + 1) * EXP_GROUP, :],
                            ps[:], Exp, scale=float(SCALE))

                    # ---- denominator: sum_j t (DVE pair-add + PE ones-mm) ----
                    dt1 = dtree.tile([128, NJ // 2, IB], f16, tag="dt")
                    nc.vector.tensor_tensor(out=dt1[:, 0:4, :],
                                            in0=t_buf[:, 0:4, :],
                                            in1=t_buf[:, 8:12, :], op=add)
                    nc.vector.tensor_tensor(out=dt1[:, 4:8, :],
                                            in0=t_buf[:, 4:8, :],
                                            in1=t_buf[:, 12:16, :], op=add)
                    nc.vector.tensor_tensor(out=dt1[:, 0:4, :],
                                            in0=dt1[:, 0:4, :],
                                            in1=dt1[:, 4:8, :], op=add)
                    pd = ps_denom.tile([128, IB], f32, tag="denom")
                    for m in range(NJ // 4):
                        nc.tensor.matmul(pd[:], lhsT=ones_bf[:],
                                         rhs=dt1[:, m, :],
                                         start=(m == 0), stop=(m == NJ // 4 - 1))
                    recip = small.tile([128, IB], f32, tag="recip")
                    nc.vector.reciprocal_approx_fast(out=recip[:], in_=pd[:])

                    # ---- dropout: t *= mask ----
                    m_buf = mpool.tile([128, NJ, IB], f16, tag="m")
                    nc.sync.dma_start(
                        out=m_buf[:],
                        in_=mask_d[s, :, isl].rearrange(
                            "(jn jp) i -> jp jn i", jp=128))
                    tm_buf = tmpool.tile([128, NJ, IB], f16, tag="tm")
                    nc.vector.tensor_tensor(out=tm_buf[:, 0:8, :],
                                            in0=t_buf[:, 0:8, :],
                                            in1=m_buf[:, 0:8, :], op=mult)
                    nc.vector.tensor_tensor(out=tm_buf[:, 8:16, :],
                                            in0=t_buf[:, 8:16, :],
                                            in1=m_buf[:, 8:16, :], op=mult)

                    # ---- PV: outT[d, i] = sum_j v[j,d] tmask[j,i] ----
                    po = ps_out.tile([128, IB], f32, tag="po")
                    for jn in range(NJ):
                        nc.tensor.matmul(po[:], lhsT=v_sb[:, jn, :],
                                         rhs=tm_buf[:, jn, :],
                                         start=(jn == 0), stop=(jn == NJ - 1))
                    nc.vector.tensor_tensor(out=outT[:, isl], in0=po[:],
                                            in1=recip[:], op=mult)

                nc.sync.dma_start(out=out_d[s], in_=outT[:])

    nc.finalize()
    return nc


def _get_program():
    global _PROGRAM
    if _PROGRAM is None:
        _PROGRAM = _build_program()
    return _PROGRAM


def _dropout_mask():
    """Exact replica of the reference's threefry dropout keep-mask."""
    import jax

    with jax.default_device(jax.devices("cpu")[0]):
        keep = jax.random.bernoulli(jax.random.key(42), 1.0 - DROPOUT_P,
                                    (B, H, S, S))
        return np.asarray(keep)


def _prepare_in_maps(query, key, value, Wq, bq, Wk, bk, Wv, bv):
    import ml_dtypes

    query = np.asarray(query, dtype=np.float32)
    key = np.asarray(key, dtype=np.float32)
    value = np.asarray(value, dtype=np.float32)
    Wq = np.asarray(Wq, dtype=np.float32)
    Wk = np.asarray(Wk, dtype=np.float32)
    Wv = np.asarray(Wv, dtype=np.float32)
    bq = np.asarray(bq, dtype=np.float32).reshape(D, 1)
    bk = np.asarray(bk, dtype=np.float32).reshape(D, 1)
    bv = np.asarray(bv, dtype=np.float32).reshape(1, D)

    keep = _dropout_mask()  # [B, H, S(i), S(j)] bool

    # per-slice transposed views: x[g] = input[b, :, h, :].T  -> [D, S]
    # g = b*H + h ; core c gets g in [8c, 8c+8)
    def xt(x):  # [B, S, H, D] -> [B*H, D, S] in fp16
        return np.ascontiguousarray(
            x.transpose(0, 2, 3, 1).reshape(B * H, D, S)).astype(np.float16)

    xqT, xkT, xvT = xt(query), xt(key), xt(value)
    maskT = np.ascontiguousarray(
        keep.transpose(0, 1, 3, 2).reshape(B * H, S, S)).astype(np.float16)

    ones_row_h = np.ones((1, IB), dtype=np.float16)
    wqT = np.ascontiguousarray(Wq.T).astype(np.float16)
    wkT = np.ascontiguousarray(Wk.T).astype(np.float16)
    # dropout 1/(1-p) folded into the V projection
    wvT = (np.ascontiguousarray(Wv.T) / (1.0 - DROPOUT_P)).astype(np.float16)
    bv = (bv / (1.0 - DROPOUT_P)).astype(np.float16)
    bq = bq.astype(np.float16)
    bk = bk.astype(np.float16)

    in_maps = []
    for c in range(N_CORES):
        g = slice(c * SLICES_PER_CORE, (c + 1) * SLICES_PER_CORE)
        in_maps.append({
            "xqT": xqT[g], "xkT": xkT[g], "xvT": xvT[g],
            "wqT": wqT, "wkT": wkT, "wvT": wvT,
            "bq": bq, "bk": bk, "bv": bv,
            "maskT": maskT[g],
            "ones_row": ones_row_h,
        })

    return in_maps


def _gather(results):
    out = np.empty((B, H, S, D), dtype=np.float32)
    for c in range(N_CORES):
        outT = results[c]["outT"]  # [8, D, S]
        for s in range(SLICES_PER_CORE):
            g = c * SLICES_PER_CORE + s
            out[g // H, g % H] = outT[s].T
    return out


def kernel(query, key, value, Wq, bq, Wk, bk, Wv, bv):
    from concourse.bass_utils import run_bass_kernel_spmd

    in_maps = _prepare_in_maps(query, key, value, Wq, bq, Wk, bk, Wv, bv)
    nc = _get_program()
    res = run_bass_kernel_spmd(nc, in_maps, list(range(N_CORES)))
    return _gather(res.results)
